# revision 9
# baseline (speedup 1.0000x reference)
"""Trainium2 Bass kernel for a seq2seq decoder step (Bahdanau attention +
LSTM cell + vocab projection), SPMD over 8 NeuronCores.

Sharding:
  - attention: data-parallel over batch (8 rows per core), encoder_output
    is the only large activation and is read once per core (67 MB).
  - LSTM: tensor-parallel over units (128 units x 4 gates per core).
  - fc: tensor-parallel over vocab (4000 cols per core).
  - two AllGathers glue the stages (context vectors, new hidden state).

Key kernel choices:
  - keys = enc @ W2 runs in bf16 on the PE at 1 cyc/row. enc tiles are
    cast fp32->bf16 during the HBM DMA (SWDGE) and transposed to
    [d, s] layout with the XBAR DMA-transpose (bf16-only HW path).
  - score = V . tanh(q + keys): tanh on ACT with the q bias fused in
    (per-partition bias), then a PE matmul whose stationary operand is V
    replicated across columns, so the score rows come out broadcast
    across partitions -- which feeds both softmax (free-dim reductions
    only) and the context reduction without any partition broadcast.
  - softmax needs no max subtraction (|score| <~ 6 in this model).
  - context = sum_s p[s] * enc[s, :] via fused DVE tensor_tensor_reduce
    on the transposed enc tiles -- no second pass over enc.
  - tail matmuls (z, fc) use float32r (full PE rate at N>=256).
"""

import sys

sys.path.insert(0, "/opt/trn_rl_repo")

import numpy as np

import concourse.bass as bass
import concourse.bacc as bacc
import concourse.tile as tile
from concourse import mybir
from concourse.bass_utils import run_bass_kernel_spmd

F32 = mybir.dt.float32
F32R = mybir.dt.float32r
BF16 = mybir.dt.bfloat16
I32 = mybir.dt.int32
AF = mybir.ActivationFunctionType
ALU = mybir.AluOpType
AX = mybir.AxisListType

B, S, D, E, V = 64, 2048, 1024, 512, 32000
NCORES = 8
BP = B // NCORES          # batch rows per core (attention shard)
US = D // NCORES          # units per core (LSTM shard)
VS = V // NCORES          # vocab cols per core (fc shard)
SC = 512                  # seq chunk
DT = D // 128             # 8 d-tiles
XT_DIMS = (D + E + D) // 128   # 20 = ctx(8) + ex(4) + hidden(8) dim-tiles
VC = 500                  # fc free-dim chunk (8 chunks of 500 = 4000)


def r32(ap):
    return ap.bitcast(F32R)


def build_program(s_len=S):
    nsc = s_len // SC
    nc = bacc.Bacc(
        "TRN2", target_bir_lowering=False, debug=False,
        enable_asserts=False, num_devices=NCORES,
    )

    # ---- external inputs (per-core contents differ; same names/shapes) ----
    enc = nc.dram_tensor("enc", [BP, s_len, D], F32, kind="ExternalInput").ap()
    hid_all = nc.dram_tensor("hid_all", [B, D], F32, kind="ExternalInput").ap()
    hid_own = nc.dram_tensor("hid_own", [BP, D], F32, kind="ExternalInput").ap()
    cell_sh = nc.dram_tensor("cell_sh", [B, US], F32, kind="ExternalInput").ap()
    x_idx = nc.dram_tensor("x_idx", [B, 1], I32, kind="ExternalInput").ap()
    emb = nc.dram_tensor("emb", [V, E], F32, kind="ExternalInput").ap()
    W1 = nc.dram_tensor("W1", [D, D], F32, kind="ExternalInput").ap()
    W1b = nc.dram_tensor("W1b", [D], F32, kind="ExternalInput").ap()
    W2 = nc.dram_tensor("W2", [D, D], F32, kind="ExternalInput").ap()
    W2b = nc.dram_tensor("W2b", [D], F32, kind="ExternalInput").ap()
    Vk = nc.dram_tensor("Vk", [D], F32, kind="ExternalInput").ap()
    lstm_kx = nc.dram_tensor("lstm_kx", [D + E, 4 * US], F32, kind="ExternalInput").ap()
    lstm_rkx = nc.dram_tensor("lstm_rkx", [D, 4 * US], F32, kind="ExternalInput").ap()
    lstm_bx = nc.dram_tensor("lstm_bx", [1, 4 * US], F32, kind="ExternalInput").ap()
    fck = nc.dram_tensor("fck", [D, VS], F32, kind="ExternalInput").ap()
    fcb = nc.dram_tensor("fcb", [1, VS], F32, kind="ExternalInput").ap()
    ident = nc.dram_tensor("ident", [128, 128], F32, kind="ExternalInput").ap()

    # ---- external outputs ----
    pred_o = nc.dram_tensor("pred", [B, VS], F32, kind="ExternalOutput").ap()
    h_o = nc.dram_tensor("h_out", [B, US], F32, kind="ExternalOutput").ap()
    c_o = nc.dram_tensor("c_out", [B, US], F32, kind="ExternalOutput").ap()
    attn_o = nc.dram_tensor("attn_out", [BP, s_len], F32, kind="ExternalOutput").ap()

    with tile.TileContext(nc) as tc:
        _emit(nc, tc, locals(), s_len, nsc)
    nc.compile()
    return nc


def _emit(nc, tc, t, s_len, nsc):
    enc, hid_all, hid_own, cell_sh, x_idx, emb = (
        t["enc"], t["hid_all"], t["hid_own"], t["cell_sh"], t["x_idx"], t["emb"])
    W1, W1b, W2, W2b, Vk = t["W1"], t["W1b"], t["W2"], t["W2b"], t["Vk"]
    lstm_kx, lstm_rkx, lstm_bx, fck, fcb, ident = (
        t["lstm_kx"], t["lstm_rkx"], t["lstm_bx"], t["fck"], t["fcb"], t["ident"])
    pred_o, h_o, c_o, attn_o = t["pred_o"], t["h_o"], t["c_o"], t["attn_o"]

    groups = [list(range(NCORES))]

    from contextlib import ExitStack
    stack = ExitStack()
    const = stack.enter_context(tc.tile_pool(name="const", bufs=1))
    dram = stack.enter_context(tc.tile_pool(name="dram", bufs=1, space="DRAM"))
    ps_small = stack.enter_context(
        tc.tile_pool(name="ps_small", bufs=2, space="PSUM"))

    ident_sb = const.tile([128, 128], F32)
    nc.sync.dma_start(ident_sb[:], ident[:])

    def pe_T(out_sb, in_sb, p, m):
        # out_sb[m, p] = in_sb[p, m] via PE transpose (through PSUM)
        ps = ps_small.tile([128, 128], F32, tag="tpose_ps")
        nc.tensor.transpose(ps[:m, :p], in_sb, ident_sb[:p, :p])
        nc.vector.tensor_copy(out_sb, ps[:m, :p])

    # ---------------- startup: constants, q, gather, transposes ----------
    # xT: [128, 20, 64] = transposed x_t/hidden for all 64 batch rows:
    #   dims 0-7 ctx (filled after AllGather), 8-11 ex, 12-19 hidden
    xT = const.tile([128, XT_DIMS, B], F32R)

    hid_nat = const.tile([B, D], F32)
    nc.sync.dma_start(hid_nat[:], hid_all[:])
    for dt in range(DT):
        pe_T(xT[:, 12 + dt, :], hid_nat[:, dt * 128:(dt + 1) * 128], B, 128)

    # embedding gather for all 64 rows -> exT
    idx_sb = const.tile([B, 1], I32)
    nc.sync.dma_start(idx_sb[:], x_idx[:])
    ex_nat = const.tile([B, E], F32)
    nc.gpsimd.indirect_dma_start(
        out=ex_nat[:], out_offset=None, in_=emb[:],
        in_offset=bass.IndirectOffsetOnAxis(ap=idx_sb[:, :1], axis=0),
    )
    for et in range(E // 128):
        pe_T(xT[:, 8 + et, :], ex_nat[:, et * 128:(et + 1) * 128], B, 128)

    # hidden (own rows) transposed for q
    hT_own = const.tile([128, DT, BP], F32)
    hid_own_nat = const.tile([BP, D], F32)
    nc.sync.dma_start(hid_own_nat[:], hid_own[:])
    for dt in range(DT):
        pe_T(hT_own[:, dt, :], hid_own_nat[:, dt * 128:(dt + 1) * 128], BP, 128)

    # qT[dout, dt, b] = (hidden_own @ W1 + W1_b + W2_b)^T
    w1b_sb = const.tile([128, DT], F32)
    nc.sync.dma_start(w1b_sb[:], W1b.rearrange("(t p) -> p t", p=128))
    w2b_sb = const.tile([128, DT], F32)
    nc.sync.dma_start(w2b_sb[:], W2b.rearrange("(t p) -> p t", p=128))
    qT = const.tile([128, DT, BP], F32)
    with tc.tile_pool(name="w1pool", bufs=2) as w1pool, \
         tc.tile_pool(name="ps_q", bufs=2, space="PSUM") as ps_q:
        for dt in range(DT):
            w1t = w1pool.tile([128, DT, 128], F32)
            nc.sync.dma_start(
                w1t[:], W1[:, dt * 128:(dt + 1) * 128].rearrange(
                    "(t p) m -> p t m", p=128))
            psq = ps_q.tile([128, BP], F32)
            for dnt in range(DT):
                nc.tensor.matmul(psq[:], w1t[:, dnt, :], hT_own[:, dnt, :],
                                 start=(dnt == 0), stop=(dnt == DT - 1))
            nc.vector.tensor_scalar(
                out=qT[:, dt, :], in0=psq[:],
                scalar1=w1b_sb[:, dt:dt + 1], scalar2=w2b_sb[:, dt:dt + 1],
                op0=ALU.add, op1=ALU.add)

    # W2 (bf16, stationary layout) + V replicated stationary
    w2_sb = const.tile([128, DT, D], BF16)
    nc.gpsimd.dma_start(w2_sb[:], W2.rearrange("(t p) m -> p t m", p=128))
    vcols = const.tile([128, DT], F32)
    nc.sync.dma_start(vcols[:], Vk.rearrange("(t p) -> p t", p=128))
    vrep = const.tile([128, D], BF16)
    for dt in range(DT):
        nc.vector.tensor_copy(
            vrep[:, dt * 128:(dt + 1) * 128],
            vcols[:, dt:dt + 1].to_broadcast([128, 128]))

    ones_f = const.tile([1, 128], F32)
    nc.vector.memset(ones_f[:], 1.0)
    ones_sb = const.tile([1, 128], F32R)
    nc.vector.tensor_copy(ones_sb[:], ones_f[:])

    cell_sb = const.tile([B, US], F32)
    nc.sync.dma_start(cell_sb[:], cell_sh[:])
    lb_sb = const.tile([1, 4 * US], F32R)
    nc.gpsimd.dma_start(lb_sb[:], lstm_bx[:])
    fcb_sb = const.tile([1, VS], F32R)
    nc.gpsimd.dma_start(fcb_sb[:], fcb[:])

    # ---------------- main attention loop ----------------
    # per-(chunk) partial context sums; reduced over chunks at the end
    # (tensor_tensor_reduce is broken on this runtime, so mult + reduce)
    ctx_parts = const.tile([128, nsc, DT, BP], F32)
    p_dram = dram.tile([BP, s_len], F32)       # unnormalized softmax numerators

    # bf16 copy of enc in DRAM (the XBAR transpose path is bf16-only and
    # reads DRAM); cast is done by SWDGE during the DRAM->DRAM copy.
    enc_bf = dram.tile([BP, s_len, D], BF16)
    for b in range(BP):
        nc.gpsimd.dma_start(enc_bf[b], enc[b])

    with tc.tile_pool(name="encT", bufs=2) as encT_pool, \
         tc.tile_pool(name="tanh", bufs=2) as tanh_pool, \
         tc.tile_pool(name="prep", bufs=2) as p_pool, \
         tc.tile_pool(name="ttr", bufs=2) as ttr_pool, \
         tc.tile_pool(name="ps_keys", bufs=2, space="PSUM") as ps_keys, \
         tc.tile_pool(name="ps_score", bufs=2, space="PSUM") as ps_score:
        for b in range(BP):
            for sc in range(nsc):
                encT = encT_pool.tile([128, DT, SC], BF16)
                for dnt in range(DT):
                    nc.sync.dma_start(
                        encT[:, dnt, :],
                        enc_bf[b, sc * SC:(sc + 1) * SC,
                               dnt * 128:(dnt + 1) * 128],
                        transpose=True)
                tanh_sb = tanh_pool.tile([128, DT, SC], BF16)
                for dt in range(DT):
                    pk = ps_keys.tile([128, SC], F32)
                    for dnt in range(DT):
                        nc.tensor.matmul(
                            pk[:], w2_sb[:, dnt, dt * 128:(dt + 1) * 128],
                            encT[:, dnt, :],
                            start=(dnt == 0), stop=(dnt == DT - 1))
                    nc.scalar.activation(
                        tanh_sb[:, dt, :], pk[:], AF.Tanh,
                        bias=qT[:, dt, b:b + 1])
                ps = ps_score.tile([128, SC], F32)
                for dt in range(DT):
                    nc.tensor.matmul(
                        ps[:], vrep[:, dt * 128:(dt + 1) * 128],
                        tanh_sb[:, dt, :],
                        start=(dt == 0), stop=(dt == DT - 1))
                p_rep = p_pool.tile([128, SC], BF16)
                nc.scalar.activation(p_rep[:], ps[:], AF.Exp)
                nc.gpsimd.dma_start(
                    p_dram[b, sc * SC:(sc + 1) * SC], p_rep[0:1, :])
                for dt in range(DT):
                    scr = ttr_pool.tile([128, SC], BF16)
                    nc.vector.tensor_tensor(
                        out=scr[:], in0=encT[:, dt, :], in1=p_rep[:],
                        op=ALU.mult)
                    nc.vector.reduce_sum(
                        ctx_parts[:, sc, dt, b:b + 1], scr[:], axis=AX.X)

    # ---------------- softmax normalization + attn output ----------------
    p_all = const.tile([BP, s_len], F32)
    nc.sync.dma_start(p_all[:], p_dram[:])
    l_sb = const.tile([BP, 1], F32)
    nc.vector.reduce_sum(l_sb[:], p_all[:], axis=AX.X)
    rl = const.tile([BP, 1], F32)
    nc.vector.reciprocal(rl[:], l_sb[:])
    attn_sb = const.tile([BP, s_len], F32)
    nc.vector.tensor_scalar_mul(attn_sb[:], p_all[:], rl[:, 0:1])
    nc.sync.dma_start(attn_o[:], attn_sb[:])

    # scale ctx columns by 1/l (per own-batch row)
    rl_row = const.tile([1, BP], F32)
    pe_T(rl_row[:], rl[:, 0:1], BP, 1)
    rl_pat = const.tile([1, DT, BP], F32)
    nc.vector.tensor_copy(
        rl_pat[:],
        rl_row.rearrange("p (x b) -> p x b", x=1).to_broadcast([1, DT, BP]))
    rl_rep = const.tile([128, DT, BP], F32)
    nc.gpsimd.partition_broadcast(rl_rep[:], rl_pat[:])
    ctx = const.tile([128, DT * BP], F32)
    nc.vector.reduce_sum(
        ctx[:], ctx_parts.rearrange("p s t b -> p t b s"), axis=AX.X)
    ctx_sc = const.tile([128, DT * BP], F32)
    nc.vector.tensor_tensor(
        out=ctx_sc[:], in0=ctx[:],
        in1=rl_rep.rearrange("p t b -> p (t b)"), op=ALU.mult)

    # ---------------- AllGather ctx -> xT[:, 0:8, :] ----------------
    ctx_loc = dram.tile([128, DT * BP], F32)
    ctx_gth = dram.tile([NCORES, 128, DT * BP], F32)
    nc.sync.dma_start(ctx_loc[:], ctx_sc[:])
    nc.gpsimd.collective_compute(
        "AllGather", ALU.bypass, replica_groups=groups,
        ins=[ctx_loc.opt()], outs=[ctx_gth.opt()])
    # xT[p, dt, (c, b)] = ctx_gth[c, p, dt, b]
    nc.gpsimd.dma_start(
        xT[:, 0:DT, :].rearrange("p t (c b) -> p t c b", c=NCORES),
        ctx_gth.rearrange("c p (t b) -> p t c b", t=DT))

    # ---------------- LSTM (unit-sharded) ----------------
    with tc.tile_pool(name="lkpool", bufs=4) as lkpool, \
         tc.tile_pool(name="ps_z", bufs=1, space="PSUM") as ps_z:
        psz = ps_z.tile([B, 4 * US], F32)
        nc.tensor.matmul(psz[:], ones_sb[0:1, 0:B], lb_sb[:],
                         start=True, stop=False)
        for dt in range(XT_DIMS):
            lk = lkpool.tile([128, 4 * US], F32R)
            if dt < 12:
                nc.gpsimd.dma_start(lk[:], lstm_kx[dt * 128:(dt + 1) * 128, :])
            else:
                nc.gpsimd.dma_start(
                    lk[:], lstm_rkx[(dt - 12) * 128:(dt - 11) * 128, :])
            nc.tensor.matmul(psz[:], xT[:, dt, :], lk[:],
                             start=False, stop=(dt == XT_DIMS - 1))

        gi = const.tile([B, US], F32)
        gf = const.tile([B, US], F32)
        gg = const.tile([B, US], F32)
        go = const.tile([B, US], F32)
        nc.scalar.activation(gi[:], psz[:, 0 * US:1 * US], AF.Sigmoid)
        nc.scalar.activation(gf[:], psz[:, 1 * US:2 * US], AF.Sigmoid)
        nc.scalar.activation(gg[:], psz[:, 2 * US:3 * US], AF.Tanh)
        nc.scalar.activation(go[:], psz[:, 3 * US:4 * US], AF.Sigmoid)

    c_new = const.tile([B, US], F32)
    nc.vector.tensor_tensor(out=c_new[:], in0=gf[:], in1=cell_sb[:], op=ALU.mult)
    ig = const.tile([B, US], F32)
    nc.vector.tensor_tensor(out=ig[:], in0=gi[:], in1=gg[:], op=ALU.mult)
    nc.vector.tensor_add(out=c_new[:], in0=c_new[:], in1=ig[:])
    tc_sb = const.tile([B, US], F32)
    nc.scalar.activation(tc_sb[:], c_new[:], AF.Tanh)
    h_new = const.tile([B, US], F32)
    nc.vector.tensor_tensor(out=h_new[:], in0=go[:], in1=tc_sb[:], op=ALU.mult)
    nc.sync.dma_start(c_o[:], c_new[:])
    nc.sync.dma_start(h_o[:], h_new[:])

    # ---------------- AllGather h^T -> fc ----------------
    hT_sb = const.tile([128, B], F32)
    pe_T(hT_sb[:], h_new[:], B, US)
    h_loc = dram.tile([128, B], F32)
    h_gth = dram.tile([NCORES, 128, B], F32)
    nc.sync.dma_start(h_loc[:], hT_sb[:])
    nc.gpsimd.collective_compute(
        "AllGather", ALU.bypass, replica_groups=groups,
        ins=[h_loc.opt()], outs=[h_gth.opt()])
    hT_full = const.tile([128, DT, B], F32R)
    nc.gpsimd.dma_start(
        hT_full[:], h_gth.rearrange("c p b -> p c b"))

    with tc.tile_pool(name="fckpool", bufs=4) as fckpool, \
         tc.tile_pool(name="ps_fc", bufs=2, space="PSUM") as ps_fc, \
         tc.tile_pool(name="predpool", bufs=2) as predpool:
        for vc in range(VS // VC):
            psf = ps_fc.tile([B, VC], F32)
            nc.tensor.matmul(
                psf[:], ones_sb[0:1, 0:B],
                fcb_sb[0:1, vc * VC:(vc + 1) * VC],
                start=True, stop=False)
            for dt in range(DT):
                fkt = fckpool.tile([128, VC], F32R)
                nc.gpsimd.dma_start(
                    fkt[:], fck[dt * 128:(dt + 1) * 128, vc * VC:(vc + 1) * VC])
                nc.tensor.matmul(psf[:], hT_full[:, dt, :], fkt[:],
                                 start=False, stop=(dt == DT - 1))
            pred_sb = predpool.tile([B, VC], F32)
            nc.scalar.copy(pred_sb[:], psf[:])
            nc.sync.dma_start(pred_o[:, vc * VC:(vc + 1) * VC], pred_sb[:])

    stack.close()


# ---------------------------------------------------------------------------
# host side
# ---------------------------------------------------------------------------

_PROGRAM_CACHE = {}


def _get_program(s_len=S):
    if s_len not in _PROGRAM_CACHE:
        _PROGRAM_CACHE[s_len] = build_program(s_len)
    return _PROGRAM_CACHE[s_len]


def make_in_maps(x, hidden, cell, encoder_output, W1_k, W1_b, W2_k, W2_b,
                 V_k, V_b, emb, lstm_k, lstm_rk, lstm_b, fc_k, fc_b):
    f = lambda a: np.ascontiguousarray(np.asarray(a), dtype=np.float32)
    x_i = np.ascontiguousarray(np.asarray(x).reshape(B, 1).astype(np.int32))
    hidden, cell, enc = f(hidden), f(cell), f(encoder_output)
    W1_k, W1_b, W2_k, W2_b, V_k = f(W1_k), f(W1_b), f(W2_k), f(W2_b), f(V_k)
    emb_f, lstm_k, lstm_rk, lstm_b = f(emb), f(lstm_k), f(lstm_rk), f(lstm_b)
    fc_k, fc_b = f(fc_k), f(fc_b)
    ident = np.eye(128, dtype=np.float32)
    vk_flat = np.ascontiguousarray(V_k.reshape(D))

    in_maps = []
    for c in range(NCORES):
        bs = slice(c * BP, (c + 1) * BP)
        us = [slice(g * D + c * US, g * D + (c + 1) * US) for g in range(4)]
        vs = slice(c * VS, (c + 1) * VS)
        in_maps.append({
            "enc": enc[bs],
            "hid_all": hidden,
            "hid_own": np.ascontiguousarray(hidden[bs]),
            "cell_sh": np.ascontiguousarray(cell[:, c * US:(c + 1) * US]),
            "x_idx": x_i,
            "emb": emb_f,
            "W1": W1_k, "W1b": W1_b, "W2": W2_k, "W2b": W2_b,
            "Vk": vk_flat,
            "lstm_kx": np.ascontiguousarray(
                np.concatenate([lstm_k[:, u] for u in us], axis=1)),
            "lstm_rkx": np.ascontiguousarray(
                np.concatenate([lstm_rk[:, u] for u in us], axis=1)),
            "lstm_bx": np.ascontiguousarray(
                np.concatenate([lstm_b[u] for u in us]).reshape(1, 4 * US)),
            "fck": np.ascontiguousarray(fc_k[:, vs]),
            "fcb": np.ascontiguousarray(fc_b[vs].reshape(1, VS)),
            "ident": ident,
        })
    return in_maps


def assemble_outputs(results):
    # LSTM gate-unit shards: core c produced gate slices for units
    # [c*US, (c+1)*US) -> concat along axis 1 in core order.
    prediction = np.concatenate([r["pred"] for r in results], axis=1)
    h_new = np.concatenate([r["h_out"] for r in results], axis=1)
    c_new = np.concatenate([r["c_out"] for r in results], axis=1)
    attn = np.concatenate([r["attn_out"] for r in results], axis=0)
    return (prediction.astype(np.float32),
            h_new.astype(np.float32),
            c_new.astype(np.float32),
            attn.reshape(B, S, 1).astype(np.float32))


def kernel(**inputs):
    nc = _get_program(S)
    in_maps = make_in_maps(**inputs)
    res = run_bass_kernel_spmd(nc, in_maps, core_ids=list(range(NCORES)))
    return assemble_outputs(res.results)


if __name__ == "__main__":
    # smoke build
    build_program(512)
    print("build ok")
